# revision 6
# baseline (speedup 1.0000x reference)
"""Trainium2 Bass kernel for nn_CubicalModel_ISM.

Reference computes Xp = X @ p and Yp = Y @ p (X, Y: [784, 32768] f32,
p: [32768] f32) and gathers 100 (i, j) positions from each reshaped
[28, 28] image.  Only the gathered rows matter: inds1/inds2 give <=100
unique rows of X and of Y (R = n1 + n2 ~ 187 of 1568 total), so the
device only needs R dot products of length 32768.

Sharding: q (parameter) axis split across 8 NeuronCores, 4096 q/core.

Layout (v2): q lives on the PARTITION axis.  Per core the host packs
sel[128, 32*R] bf16 where column block j holds [X|Y]-rows for q-chunk
j (sel[p, j*R + r] = row_r[q = j*128 + p]).  All 128 partitions carry
useful bytes, so the HBM stream runs at the full ~358 GB/s/core port
rate (the old [nr~94, 8192] f32 layout idled 34/128 SBUF ports and got
~240 GB/s), and bf16 halves the bytes: ~0.75 MB/core vs 3 MB.

Precision: a raw bf16 cast fails (max rel err 0.25 vs the 2e-2 gate;
min gathered |dot| ~ 1.2 while the bf16 rounding noise of a 32k-term
dot is ~0.25).  Fix: the host applies the same permutation to the q
axis of X/Y and p that sorts p ascending (a pure reindexing - the dot
products are permutation invariant), then quantizes each row with an
error-feedback carry chain along the sorted axis, running toward the
smallest-|p| end of each core's shard.  The quantization error of the
dot then telescopes to sum_i carry_i * (p_i - p_{i-1}) over the tiny
sorted gaps plus one end-of-chain term at the shard's smallest |p|:
measured max rel err 1.5e-3 (vs 3.7e-4 for the all-f32 baseline).

Compute: PE matvec.  p ships as bf16 hi/lo halves (hi+lo rebuilds p to
~2^-18); per q-chunk j the stationary operand is ph[:, 2j:2j+2] =
[p_hi_j | p_lo_j] (LDWEIGHTS cost scales with stationary COLUMNS = 2,
~2 cycles) and the moving operand is sel's block j (R bf16 columns, 1
col/cycle).  32 chunk matmuls accumulate into one PSUM region [2, R]
f32 (row 0 = hi dot, row 1 = lo dot; host adds them).  DVE copies
[2, R] PSUM->SBUF and a 2-line DMA returns it.  The host sums the 8
per-core partials (the all-reduce) and applies the unique-inverse
gather.
"""

import numpy as np

H = W = 28
Q = 32768
N_CORES = 8
QS = Q // N_CORES   # 4096 q per core
NJ = QS // 128      # 32 q-chunks of 128 (PE contraction dim)
BPP = 4             # q-chunks per DMA piece
NP = NJ // BPP      # 8 DMA pieces per core
PHW = 2 * NJ        # 64 leading sel columns carry p hi/lo (folded into piece 0)

_CACHE = {}


def _build_nc(R):
    import concourse.bacc as bacc
    import concourse.mybir as mybir
    from concourse.tile import TileContext

    nc = bacc.Bacc(None, enable_partition_id=False)
    f32 = mybir.dt.float32
    bf16 = mybir.dt.bfloat16
    # sel columns: [ph hi/lo (64) | block 0 (R) | ... | block 31 (R)].
    # Folding ph into the head of piece 0 avoids a separate 128x128B
    # tiny-packet DMA (the [128, 64] line length is below the 512B
    # descriptor floor) and one ~0.6us HWDGE issue slot.
    sel = nc.dram_tensor("sel", [128, PHW + NJ * R], bf16, kind="ExternalInput")
    out = nc.dram_tensor("out", [2, R], f32, kind="ExternalOutput")

    PW = BPP * R  # stream columns per DMA piece

    with TileContext(nc) as tc:
        with (
            tc.tile_pool(name="pieces", bufs=1) as piece_pool,
            tc.tile_pool(name="respool", bufs=1) as res_pool,
            tc.tile_pool(name="psum", bufs=1, space="PSUM") as psum_pool,
        ):
            # piece 0 carries ph + blocks 0..3; pieces 1..7 carry 4 blocks
            # each.  8 equal-ish pieces: PE (cold ~182 ns/chunk) tracks the
            # ~255 GB/s stream (~190 ns/chunk of data) with piece-granular
            # sem waits, so smaller pieces cut the end-of-stream lag.
            pieces = [
                piece_pool.tile(
                    [128, (PHW if k == 0 else 0) + PW],
                    bf16,
                    tag=f"piece{k}",
                    name=f"piece{k}",
                )
                for k in range(NP)
            ]
            off = 0
            for k in range(NP):
                w = pieces[k].shape[1]
                nc.sync.dma_start(out=pieces[k][:, :], in_=sel[:, off : off + w])
                off += w
            acc = psum_pool.tile([2, R], f32)
            for j in range(NJ):
                k, jj = divmod(j, BPP)
                roff = (PHW if k == 0 else 0) + jj * R
                nc.tensor.matmul(
                    acc[:, :],
                    pieces[0][:, 2 * j : 2 * j + 2],
                    pieces[k][:, roff : roff + R],
                    start=(j == 0),
                    stop=(j == NJ - 1),
                )
            res = res_pool.tile([2, R], f32)
            nc.vector.tensor_copy(res[:, :], acc[:, :])
            nc.sync.dma_start(out=out[:, :], in_=res[:, :])
    nc.finalize()
    return nc


def _get_nc(R):
    if R not in _CACHE:
        _CACHE[R] = _build_nc(R)
    return _CACHE[R]


def _unique_rows(inds):
    # inds: [200] int pairs (i, j); flat row index i*28 + j into the
    # row-major [784]-row matvec output.
    ij = np.asarray(inds).reshape(-1, 2).astype(np.int64)
    flat = ij[:, 0] * W + ij[:, 1]
    return np.unique(flat, return_inverse=True)


def _feedback_quant(M, ps, bf16):
    """Quantize M [R, Q] (columns already in sorted-p order) to bf16 with
    per-(row, core-shard) error-feedback carry chains.  Each chain runs
    toward the smallest-|p| end of its shard so the dropped end carry
    multiplies the smallest available |p|."""
    R = M.shape[0]
    out = np.empty((R, Q), dtype=bf16)
    for s in range(N_CORES):
        lo, hi = s * QS, (s + 1) * QS
        seg = M[:, lo:hi]
        idx = range(QS) if ps[lo] + ps[hi - 1] < 0 else range(QS - 1, -1, -1)
        carry = np.zeros(R, dtype=np.float32)
        oseg = np.empty((R, QS), dtype=bf16)
        for j in idx:
            t = seg[:, j] + carry
            q = t.astype(bf16)
            carry = t - q.astype(np.float32)
            oseg[:, j] = q
        out[:, lo:hi] = oseg
    return out


def _prep(X, Y, p, inds1, inds2):
    """Host prep: unique-row selection, p-sort, feedback quantization,
    per-core transposed packing.  Returns (nc, in_maps, meta)."""
    import ml_dtypes

    bf16 = ml_dtypes.bfloat16
    X = np.asarray(X, dtype=np.float32)
    Y = np.asarray(Y, dtype=np.float32)
    p = np.asarray(p, dtype=np.float32)

    u1, inv1 = _unique_rows(inds1)
    u2, inv2 = _unique_rows(inds2)
    n1, n2 = len(u1), len(u2)
    R = n1 + n2

    p_hi16 = p.astype(bf16)
    p_hi = p_hi16.astype(np.float32)
    p_lo16 = (p - p_hi).astype(bf16)
    p_rec = p_hi + p_lo16.astype(np.float32)

    order = np.argsort(p_rec, kind="stable")
    ps = p_rec[order]
    hi_s = p_hi16[order]
    lo_s = p_lo16[order]

    M = np.concatenate([X[u1], Y[u2]], axis=0)[:, order]  # [R, Q] sorted-q
    Mq = _feedback_quant(M, ps, bf16)

    in_maps = []
    for c in range(N_CORES):
        sh = Mq[:, c * QS : (c + 1) * QS]  # [R, 4096] bf16
        buf = np.empty((128, PHW + NJ * R), dtype=bf16)
        buf[:, 0:PHW:2] = hi_s[c * QS : (c + 1) * QS].reshape(NJ, 128).T
        buf[:, 1:PHW:2] = lo_s[c * QS : (c + 1) * QS].reshape(NJ, 128).T
        buf[:, PHW:] = sh.reshape(R, NJ, 128).transpose(2, 1, 0).reshape(
            128, NJ * R
        )
        in_maps.append({"sel": buf})

    nc = _get_nc(R)
    return nc, in_maps, (n1, n2, inv1, inv2, R)


def kernel(X, Y, p, inds1, inds2):
    from concourse.bass_utils import run_bass_kernel_spmd

    nc, in_maps, (n1, n2, inv1, inv2, R) = _prep(X, Y, p, inds1, inds2)
    results = run_bass_kernel_spmd(
        nc, in_maps, list(range(N_CORES))
    ).results

    total = np.zeros(R, dtype=np.float32)
    for c in range(N_CORES):
        o = results[c]["out"]  # [2, R]: hi dot, lo dot
        total += o[0] + o[1]

    dgm1 = total[:n1][inv1].reshape(-1, 2).astype(np.float32, copy=False)
    dgm2 = total[n1:][inv2].reshape(-1, 2).astype(np.float32, copy=False)
    return dgm1, dgm2
